# revision 1
# baseline (speedup 1.0000x reference)
"""Trainium2 Bass kernel for the Context Encoder problem:

    ce  = c2e_weight[nodes]            # [N, 128] embedding gather
    h   = relu(ce @ w1.T + b1)         # [N, 128]
    out = relu(h @ w2.T + b2)          # [N, 128]

Strategy (8 NeuronCores, vocab-range sharding):
  200000 node ids over a 100000-row vocab saturate every vocab window,
  so transforming the table itself is less work than gathering per-node
  rows (and avoids the per-index DMA descriptor-generation cost that
  dominates any on-device gather).

  - The vocab is split into 8 fixed 12500-row ranges.  Core i streams
    its host-pre-transposed (d-major) table window [128, 12800]
    contiguously at full DMA bandwidth and computes
    T2 = relu(relu(win @ w1.T + b1) @ w2.T + b2) for every window row.
  - d-major input feeds mm1 directly (lhsT = w1.T stationary, window as
    the moving operand); mm2 keeps w2.T stationary.  Both layers' biases
    are per-partition, so relu+bias fuses into one ScalarE activation or
    one VectorE dual-op tensor_scalar; the two relus alternate between
    ACT and DVE for engine balance.  No PE transposes, no PSUM staging
    copies, no bias matmuls.
  - Results stay feature-major; the host transposes each window and maps
    node positions to rows (out = T2[nodes]) as the unshard step.
"""

import os
import sys

for _p in ("/opt/trn_rl_repo",):
    if _p not in sys.path:
        sys.path.insert(0, _p)

import numpy as np

import concourse.bass as bass
import concourse.mybir as mybir
import concourse.tile as tile
from concourse import bacc
from concourse.bass_utils import run_bass_kernel_spmd
from concourse.tile import TileContext

P = 128
D = 128
N_CORES = 8
VOCAB = 100000
RANGE = VOCAB // N_CORES   # 12500 vocab rows owned per core
BLOCKS = 100               # 12800 rows processed per core (128*100)
CHUNK_BLOCKS = 20          # rows DMA'd per input chunk (1.31 MB)
G = 4                      # blocks per compute super-tile (free dim 512)


def build_nc(blocks: int = BLOCKS, chunk_blocks: int = CHUNK_BLOCKS,
             g: int = G, use_f32r: bool = False):
    assert blocks % g == 0 and chunk_blocks % g == 0
    f32 = mybir.dt.float32
    nc = bacc.Bacc("TRN2", target_bir_lowering=False, debug=False,
                   num_devices=N_CORES)

    rows = blocks * P
    tsl_t = nc.dram_tensor("tslice", [P, rows], f32,
                           kind="ExternalInput").ap()
    w1t_t = nc.dram_tensor("w1t", [D, D], f32, kind="ExternalInput").ap()
    w2t_t = nc.dram_tensor("w2t", [D, D], f32, kind="ExternalInput").ap()
    b1_t = nc.dram_tensor("b1c", [P, 1], f32, kind="ExternalInput").ap()
    b2_t = nc.dram_tensor("b2c", [P, 1], f32, kind="ExternalInput").ap()
    out_t = nc.dram_tensor("out", [P, rows], f32,
                           kind="ExternalOutput").ap()

    fw = g * D  # super-tile free width (512)

    with TileContext(nc) as tc:
        with (
            tc.tile_pool(name="const", bufs=1) as cpool,
            tc.tile_pool(name="win", bufs=3) as gpool,
            tc.tile_pool(name="work", bufs=3) as wpool,
            tc.tile_pool(name="psum", bufs=3, space="PSUM") as ppool,
        ):
            w1t_sb = cpool.tile([D, D], f32, tag="w1t")
            nc.sync.dma_start(out=w1t_sb[:], in_=w1t_t[:])
            w2t_sb = cpool.tile([D, D], f32, tag="w2t")
            nc.sync.dma_start(out=w2t_sb[:], in_=w2t_t[:])
            b1_sb = cpool.tile([P, 1], f32, tag="b1")
            nc.sync.dma_start(out=b1_sb[:], in_=b1_t[:])
            b2_sb = cpool.tile([P, 1], f32, tag="b2")
            nc.sync.dma_start(out=b2_sb[:], in_=b2_t[:])

            def relu_bias(out_ap, in_ap, bias_sb, on_act: bool):
                if on_act:
                    nc.scalar.activation(out_ap, in_ap,
                                         mybir.ActivationFunctionType.Relu,
                                         bias=bias_sb[:, 0:1])
                else:
                    nc.vector.tensor_scalar(
                        out=out_ap, in0=in_ap, scalar1=bias_sb[:, 0:1],
                        scalar2=0.0, op0=mybir.AluOpType.add,
                        op1=mybir.AluOpType.max)

            def mmcast(ap):
                return ap.bitcast(mybir.dt.float32r) if use_f32r else ap

            # small first chunk so mm1 starts as early as possible
            chunks = [g] + [chunk_blocks] * ((blocks - g) // chunk_blocks)
            rem = blocks - sum(chunks)
            assert rem % g == 0
            if rem:
                chunks.append(rem)

            st = 0
            r0 = 0
            for cb in chunks:
                win = gpool.tile([P, chunk_blocks * D], f32, tag="win")
                nc.sync.dma_start(
                    out=win[:, : cb * D], in_=tsl_t[:, r0 : r0 + cb * P])
                for s in range(cb // g):
                    r0s = r0 + s * fw
                    ceT = win[:, s * fw : (s + 1) * fw]

                    h_ps = ppool.tile([P, fw], f32, tag="h")
                    nc.tensor.matmul(out=h_ps[:], lhsT=mmcast(w1t_sb[:]),
                                     rhs=mmcast(ceT), start=True, stop=True)
                    hT_sb = wpool.tile([P, fw], f32, tag="hT")
                    relu_bias(hT_sb[:], h_ps[:], b1_sb, on_act=(st % 2 == 0))

                    o_ps = ppool.tile([P, fw], f32, tag="o")
                    nc.tensor.matmul(out=o_ps[:], lhsT=mmcast(w2t_sb[:]),
                                     rhs=mmcast(hT_sb[:]), start=True,
                                     stop=True)
                    o_sb = wpool.tile([P, fw], f32, tag="o_sb")
                    relu_bias(o_sb[:], o_ps[:], b2_sb, on_act=(st % 2 == 1))
                    st += 1

                    nc.sync.dma_start(out=out_t[:, r0s : r0s + fw],
                                      in_=o_sb[:])
                r0 += cb * P

    nc.compile()
    return nc


_CACHED_NC = None
LAST_RESULTS = None


def _get_nc():
    global _CACHED_NC
    if _CACHED_NC is None:
        _CACHED_NC = build_nc(
            use_f32r=os.environ.get("BASS_KERNEL_F32R", "0") == "1")
    return _CACHED_NC


def kernel(nodes, c2e_weight, w1, b1, w2, b2):
    nodes = np.asarray(nodes).astype(np.int64)
    c2e_weight = np.asarray(c2e_weight, dtype=np.float32)
    w1 = np.asarray(w1, dtype=np.float32)
    b1 = np.asarray(b1, dtype=np.float32)
    w2 = np.asarray(w2, dtype=np.float32)
    b2 = np.asarray(b2, dtype=np.float32)

    vocab = c2e_weight.shape[0]
    assert vocab == VOCAB, vocab
    rows = BLOCKS * P  # 12800

    tableT = np.ascontiguousarray(c2e_weight.T)  # [128, VOCAB], d-major

    w1t = np.ascontiguousarray(w1.T)
    w2t = np.ascontiguousarray(w2.T)
    b1c = np.ascontiguousarray(b1.reshape(P, 1))
    b2c = np.ascontiguousarray(b2.reshape(P, 1))

    starts = []
    in_maps = []
    for i in range(N_CORES):
        start = min(i * RANGE, vocab - rows)
        starts.append(start)
        in_maps.append({
            "tslice": np.ascontiguousarray(tableT[:, start : start + rows]),
            "w1t": w1t,
            "w2t": w2t,
            "b1c": b1c,
            "b2c": b2c,
        })

    nc = _get_nc()
    trace = os.environ.get("BASS_KERNEL_TRACE") == "1"
    if trace:
        try:  # tracing needs the NTFF hook; degrade silently without it
            import antenv.axon_hooks  # noqa: F401
        except ImportError:
            trace = False
    res = run_bass_kernel_spmd(nc, in_maps, core_ids=list(range(N_CORES)),
                               trace=trace)
    global LAST_RESULTS
    LAST_RESULTS = res

    # T2[v] = MLP(c2e_weight[v]) assembled from the 8 windows
    t2 = np.empty((vocab, D), dtype=np.float32)
    for i in range(N_CORES):
        dense = res.results[i]["out"]                    # [128, rows] (k, r)
        lo = i * RANGE
        hi = min((i + 1) * RANGE, vocab)
        t2[lo:hi] = dense[:, lo - starts[i] : hi - starts[i]].T

    return t2[nodes]



# revision 5
# speedup vs baseline: 1.3383x; 1.3383x over previous
"""Trainium2 Bass kernel for the Context Encoder problem:

    ce  = c2e_weight[nodes]            # [N, 128] embedding gather
    h   = relu(ce @ w1.T + b1)         # [N, 128]
    out = relu(h @ w2.T + b2)          # [N, 128]

Strategy (8 NeuronCores, unique-row compaction, bf16):
  200000 node ids cover ~86.5% of the 100000-row vocab, so transforming
  the table rows that are actually referenced is less total work than
  gathering per-node rows (the output of a per-node gather would be 2.3x
  larger than the compacted table).

  - Host computes used = unique(nodes) (~86.4k rows), pads to a fixed
    8*11264 = 90112 rows, gathers those table rows, downcasts to bf16
    and hands core i its contiguous d-major window [128, 11264].
  - Each core streams its window through the 2-layer MLP:
    matmuls in bf16 (1 cycle/row on PE vs 4 for fp32), fp32 PSUM
    accumulation, relu+bias fused on ACT (layer 1) / DVE (layer 2)
    draining PSUM directly, bf16 results DMA'd back out.
  - Work is chunked (1024 + 5x2048 columns); matmuls are grouped 4 per
    [128, 2048] PSUM tile (each sub-matmul targets one 512-fp32 bank)
    so each relu is a single wide op and weight reloads alternate only
    once per group.
  - Host upcasts and applies out = T2used[inverse(nodes)] as the
    unshard step.  If a pathological input references more than 90112
    distinct rows, a full-vocab variant (98 blocks/core, staggered
    windows) is built and used instead.
"""

import os
import sys

for _p in ("/opt/trn_rl_repo",):
    if _p not in sys.path:
        sys.path.insert(0, _p)

import numpy as np
import ml_dtypes

import concourse.bass as bass
import concourse.mybir as mybir
import concourse.tile as tile
from concourse import bacc
from concourse.bass_utils import run_bass_kernel_spmd
from concourse.tile import TileContext

P = 128
D = 128
N_CORES = 8
VOCAB = 100000
BLOCKS = 88                # vocab blocks (of 128 rows) per core, compact path
FULL_BLOCKS = 98           # fallback: cover the whole vocab with overlap
GROUP = 2048               # columns per PSUM group / relu op (4 banks)
FW = 512                   # matmul free width (one fp32 PSUM bank)

BF16 = ml_dtypes.bfloat16


def build_nc(blocks: int):
    f32 = mybir.dt.float32
    bf16 = mybir.dt.bfloat16
    nc = bacc.Bacc("TRN2", target_bir_lowering=False, debug=False,
                   num_devices=N_CORES)

    rows = blocks * P
    tsl_t = nc.dram_tensor("tslice", [P, rows], bf16,
                           kind="ExternalInput").ap()
    w1t_t = nc.dram_tensor("w1t", [D, D], bf16, kind="ExternalInput").ap()
    w2t_t = nc.dram_tensor("w2t", [D, D], bf16, kind="ExternalInput").ap()
    b1_t = nc.dram_tensor("b1c", [P, 1], f32, kind="ExternalInput").ap()
    b2_t = nc.dram_tensor("b2c", [P, 1], f32, kind="ExternalInput").ap()
    out_t = nc.dram_tensor("out", [P, rows], bf16,
                           kind="ExternalOutput").ap()

    # column chunks: small first chunk so compute starts early
    chunks = [GROUP // 2]
    while sum(chunks) + GROUP <= rows:
        chunks.append(GROUP)
    rem = rows - sum(chunks)
    assert rem % P == 0
    if rem:
        chunks.append(rem)

    def subtiles(cb):
        off = 0
        while off < cb:
            w = min(FW, cb - off)
            yield off, w
            off += w

    with TileContext(nc) as tc:
        with (
            tc.tile_pool(name="const", bufs=1) as cpool,
            tc.tile_pool(name="win", bufs=3) as gpool,
            tc.tile_pool(name="work", bufs=2) as wpool,
            tc.tile_pool(name="obuf", bufs=3) as opool,
            tc.tile_pool(name="psum", bufs=1, space="PSUM") as ppool,
        ):
            # constants arrive on the scalar HWDGE ring so the first window
            # chunk (sync ring) streams in parallel with them
            w1t_sb = cpool.tile([D, D], bf16, tag="w1t")
            nc.scalar.dma_start(out=w1t_sb[:], in_=w1t_t[:])
            w2t_sb = cpool.tile([D, D], bf16, tag="w2t")
            nc.scalar.dma_start(out=w2t_sb[:], in_=w2t_t[:])
            b1_sb = cpool.tile([P, 1], f32, tag="b1")
            nc.scalar.dma_start(out=b1_sb[:], in_=b1_t[:])
            b2_sb = cpool.tile([P, 1], f32, tag="b2")
            nc.scalar.dma_start(out=b2_sb[:], in_=b2_t[:])

            r0 = 0
            for cb in chunks:
                win = gpool.tile([P, GROUP], bf16, tag="win")
                nc.sync.dma_start(out=win[:, :cb], in_=tsl_t[:, r0:r0 + cb])

                h_ps = ppool.tile([P, GROUP], f32, tag="h")
                for off, w in subtiles(cb):
                    nc.tensor.matmul(out=h_ps[:, off:off + w],
                                     lhsT=w1t_sb[:],
                                     rhs=win[:, off:off + w],
                                     start=True, stop=True)
                hT = wpool.tile([P, GROUP], bf16, tag="hT")
                nc.scalar.activation(hT[:, :cb], h_ps[:, :cb],
                                     mybir.ActivationFunctionType.Relu,
                                     bias=b1_sb[:, 0:1])

                o_ps = ppool.tile([P, GROUP], f32, tag="o")
                for off, w in subtiles(cb):
                    nc.tensor.matmul(out=o_ps[:, off:off + w],
                                     lhsT=w2t_sb[:],
                                     rhs=hT[:, off:off + w],
                                     start=True, stop=True)
                ob = opool.tile([P, GROUP], bf16, tag="ob")
                nc.vector.tensor_scalar(
                    out=ob[:, :cb], in0=o_ps[:, :cb],
                    scalar1=b2_sb[:, 0:1], scalar2=0.0,
                    op0=mybir.AluOpType.add, op1=mybir.AluOpType.max)

                nc.sync.dma_start(out=out_t[:, r0:r0 + cb], in_=ob[:, :cb])
                r0 += cb

    nc.compile()
    return nc


_CACHED_NC = {}
LAST_RESULTS = None


def _get_nc(blocks: int):
    nc = _CACHED_NC.get(blocks)
    if nc is None:
        nc = _CACHED_NC[blocks] = build_nc(blocks)
    return nc


def _run(nc, in_maps):
    global LAST_RESULTS
    trace = os.environ.get("BASS_KERNEL_TRACE") == "1"
    if trace:
        try:  # tracing needs the NTFF hook; degrade silently without it
            import antenv.axon_hooks  # noqa: F401
        except ImportError:
            trace = False
    res = run_bass_kernel_spmd(nc, in_maps, core_ids=list(range(N_CORES)),
                               trace=trace)
    LAST_RESULTS = res
    return res


def kernel(nodes, c2e_weight, w1, b1, w2, b2):
    nodes = np.asarray(nodes)
    c2e_weight = np.asarray(c2e_weight, dtype=np.float32)
    w1t = np.ascontiguousarray(np.asarray(w1, dtype=np.float32).T).astype(BF16)
    w2t = np.ascontiguousarray(np.asarray(w2, dtype=np.float32).T).astype(BF16)
    b1c = np.ascontiguousarray(
        np.asarray(b1, dtype=np.float32).reshape(P, 1))
    b2c = np.ascontiguousarray(
        np.asarray(b2, dtype=np.float32).reshape(P, 1))

    vocab = c2e_weight.shape[0]
    assert vocab == VOCAB, vocab

    used, inv = np.unique(nodes, return_inverse=True)
    tot = BLOCKS * P * N_CORES
    rows = BLOCKS * P

    if len(used) <= tot:
        # compact path: transform only referenced rows
        used_pad = np.empty(tot, dtype=np.int64)
        used_pad[:len(used)] = used
        used_pad[len(used):] = int(used[-1]) if len(used) else 0
        compact = c2e_weight[used_pad].astype(BF16)   # [tot, 128]

        common = {"w1t": w1t, "w2t": w2t, "b1c": b1c, "b2c": b2c}
        in_maps = []
        for i in range(N_CORES):
            sl = compact[i * rows:(i + 1) * rows]
            in_maps.append(
                {"tslice": np.ascontiguousarray(sl.T), **common})

        res = _run(_get_nc(BLOCKS), in_maps)

        t2u = np.concatenate(
            [np.asarray(res.results[i]["out"]).T for i in range(N_CORES)],
            axis=0)                                   # [tot, 128] bf16
        return t2u[inv].astype(np.float32)

    # fallback: transform the whole vocab with staggered windows
    frows = FULL_BLOCKS * P
    tableT = np.ascontiguousarray(c2e_weight.T).astype(BF16)  # [128, vocab]
    common = {"w1t": w1t, "w2t": w2t, "b1c": b1c, "b2c": b2c}
    starts, in_maps = [], []
    rng = vocab // N_CORES
    for i in range(N_CORES):
        start = min(i * rng, vocab - frows)
        starts.append(start)
        in_maps.append(
            {"tslice": np.ascontiguousarray(tableT[:, start:start + frows]),
             **common})

    res = _run(_get_nc(FULL_BLOCKS), in_maps)

    t2 = np.empty((vocab, D), dtype=np.float32)
    for i in range(N_CORES):
        dense = np.asarray(res.results[i]["out"])     # [128, frows]
        lo = i * rng
        hi = min((i + 1) * rng, vocab)
        t2[lo:hi] = dense[:, lo - starts[i]:hi - starts[i]].T
    return t2[nodes]


# revision 6
# speedup vs baseline: 1.8371x; 1.3727x over previous
"""Trainium2 Bass kernel for the Context Encoder problem:

    ce  = c2e_weight[nodes]            # [N, 128] embedding gather
    h   = relu(ce @ w1.T + b1)         # [N, 128]
    out = relu(h @ w2.T + b2)          # [N, 128]

Strategy (8 NeuronCores, unique-row compaction, bf16):
  200000 node ids cover ~86.4% of the 100000-row vocab, so transforming
  the table rows that are actually referenced is less total work than
  gathering per-node rows (the output of a per-node gather would be 2.3x
  larger than the compacted table).

  - Host computes used = unique(nodes) (~86.4k rows), pads to a fixed
    8*10880 = 87040 rows, gathers those table rows, downcasts to bf16
    and hands core i its contiguous d-major window [128, 10880].
  - Each core streams its window through the 2-layer MLP: matmuls in
    bf16 (1 cycle/row on PE vs 4 for fp32), fp32 PSUM accumulation,
    relu+bias fused on ACT/DVE draining PSUM, bf16 results DMA'd out.
  - All input chunk DMAs are issued up-front on the sync HWDGE ring
    (the reads stream at full bandwidth, decoupled from compute);
    output DMAs ride the otherwise-idle GpSimd SWDGE ring so neither
    stream head-of-line blocks the other.  Weights/biases preload on
    the scalar HWDGE ring.
  - Within a chunk the two layers run as phases (one LDWEIGHTS each);
    matmuls are 512 wide (one fp32 PSUM bank), relu+bias ops 1024 wide
    over double-buffered 2-bank PSUM tiles, and the two relu stages
    alternate between ACT and DVE by chunk parity for balance.
  - Host upcasts and applies out = T2used[inverse(nodes)] as the
    unshard step.  If a pathological input references more than 87040
    distinct rows, a full-vocab variant (98 blocks/core, staggered
    windows) is built and used instead.
"""

import os
import sys

for _p in ("/opt/trn_rl_repo",):
    if _p not in sys.path:
        sys.path.insert(0, _p)

import numpy as np
import ml_dtypes

import concourse.bass as bass
import concourse.mybir as mybir
import concourse.tile as tile
from concourse import bacc
from concourse.bass_utils import run_bass_kernel_spmd
from concourse.tile import TileContext

P = 128
D = 128
N_CORES = 8
VOCAB = 100000
BLOCKS = 85                # vocab blocks (of 128 rows) per core, compact path
FULL_BLOCKS = 98           # fallback: cover the whole vocab with overlap
CHUNK = 2048               # columns per input chunk / out DMA
RELU_W = 1024              # columns per relu op (2 fp32 PSUM banks)
FW = 512                   # matmul free width (one fp32 PSUM bank)

BF16 = ml_dtypes.bfloat16


def build_nc(blocks: int):
    f32 = mybir.dt.float32
    bf16 = mybir.dt.bfloat16
    nc = bacc.Bacc("TRN2", target_bir_lowering=False, debug=False,
                   num_devices=N_CORES)

    rows = blocks * P
    tsl_t = nc.dram_tensor("tslice", [P, rows], bf16,
                           kind="ExternalInput").ap()
    w1t_t = nc.dram_tensor("w1t", [D, D], bf16, kind="ExternalInput").ap()
    w2t_t = nc.dram_tensor("w2t", [D, D], bf16, kind="ExternalInput").ap()
    b1_t = nc.dram_tensor("b1c", [P, 1], f32, kind="ExternalInput").ap()
    b2_t = nc.dram_tensor("b2c", [P, 1], f32, kind="ExternalInput").ap()
    out_t = nc.dram_tensor("out", [P, rows], bf16,
                           kind="ExternalOutput").ap()

    # column chunks: small first chunk so compute starts early
    chunks = [RELU_W]
    while sum(chunks) + CHUNK <= rows:
        chunks.append(CHUNK)
    rem = rows - sum(chunks)
    assert rem % P == 0
    if rem:
        chunks.append(rem)

    def pieces(cb, w):
        off = 0
        while off < cb:
            yield off, min(w, cb - off)
            off += w

    with TileContext(nc) as tc:
        with (
            tc.tile_pool(name="const", bufs=1) as cpool,
            tc.tile_pool(name="win", bufs=len(chunks)) as gpool,
            tc.tile_pool(name="work", bufs=3) as wpool,
            tc.tile_pool(name="obuf", bufs=3) as opool,
            tc.tile_pool(name="psum", bufs=2, space="PSUM") as ppool,
        ):
            # constants on the scalar HWDGE ring, in parallel with the
            # window stream on the sync ring
            w1t_sb = cpool.tile([D, D], bf16, tag="w1t")
            nc.scalar.dma_start(out=w1t_sb[:], in_=w1t_t[:])
            w2t_sb = cpool.tile([D, D], bf16, tag="w2t")
            nc.scalar.dma_start(out=w2t_sb[:], in_=w2t_t[:])
            b1_sb = cpool.tile([P, 1], f32, tag="b1")
            nc.scalar.dma_start(out=b1_sb[:], in_=b1_t[:])
            b2_sb = cpool.tile([P, 1], f32, tag="b2")
            nc.scalar.dma_start(out=b2_sb[:], in_=b2_t[:])

            # issue every input chunk DMA up-front: the sync ring streams
            # the whole window at full rate, decoupled from compute
            wins = []
            r0 = 0
            for cb in chunks:
                win = gpool.tile([P, CHUNK], bf16, tag="win")
                nc.sync.dma_start(out=win[:, :cb], in_=tsl_t[:, r0:r0 + cb])
                wins.append(win)
                r0 += cb

            def relu_bias(out_ap, in_ap, bias_sb, on_act: bool):
                if on_act:
                    nc.scalar.activation(out_ap, in_ap,
                                         mybir.ActivationFunctionType.Relu,
                                         bias=bias_sb[:, 0:1])
                else:
                    nc.vector.tensor_scalar(
                        out=out_ap, in0=in_ap, scalar1=bias_sb[:, 0:1],
                        scalar2=0.0, op0=mybir.AluOpType.add,
                        op1=mybir.AluOpType.max)

            r0 = 0
            for ci, cb in enumerate(chunks):
                win = wins[ci]
                par = ci % 2 == 0

                # layer 1 phase: one LDWEIGHTS, relu per RELU_W group
                hts = []
                for goff, gw in pieces(cb, RELU_W):
                    h_ps = ppool.tile([P, RELU_W], f32, tag="h")
                    for off, w in pieces(gw, FW):
                        nc.tensor.matmul(
                            out=h_ps[:, off:off + w],
                            lhsT=w1t_sb[:],
                            rhs=win[:, goff + off:goff + off + w],
                            start=True, stop=True)
                    hT = wpool.tile([P, RELU_W], bf16, tag="hT")
                    relu_bias(hT[:, :gw], h_ps[:, :gw], b1_sb, par)
                    hts.append((hT, gw))

                # layer 2 phase
                ob = opool.tile([P, CHUNK], bf16, tag="ob")
                for (hT, gw), (goff, _) in zip(hts, pieces(cb, RELU_W)):
                    o_ps = ppool.tile([P, RELU_W], f32, tag="o")
                    for off, w in pieces(gw, FW):
                        nc.tensor.matmul(
                            out=o_ps[:, off:off + w],
                            lhsT=w2t_sb[:],
                            rhs=hT[:, off:off + w],
                            start=True, stop=True)
                    relu_bias(ob[:, goff:goff + gw], o_ps[:, :gw], b2_sb,
                              not par)

                # out stream on the GpSimd SWDGE ring — independent of the
                # input stream's HWDGE ring, no head-of-line coupling
                nc.gpsimd.dma_start(out=out_t[:, r0:r0 + cb], in_=ob[:, :cb])
                r0 += cb

    nc.compile()
    return nc


_CACHED_NC = {}
LAST_RESULTS = None


def _get_nc(blocks: int):
    nc = _CACHED_NC.get(blocks)
    if nc is None:
        nc = _CACHED_NC[blocks] = build_nc(blocks)
    return nc


def _run(nc, in_maps):
    global LAST_RESULTS
    trace = os.environ.get("BASS_KERNEL_TRACE") == "1"
    if trace:
        try:  # tracing needs the NTFF hook; degrade silently without it
            import antenv.axon_hooks  # noqa: F401
        except ImportError:
            trace = False
    res = run_bass_kernel_spmd(nc, in_maps, core_ids=list(range(N_CORES)),
                               trace=trace)
    LAST_RESULTS = res
    return res


def kernel(nodes, c2e_weight, w1, b1, w2, b2):
    nodes = np.asarray(nodes)
    c2e_weight = np.asarray(c2e_weight, dtype=np.float32)
    w1t = np.ascontiguousarray(np.asarray(w1, dtype=np.float32).T).astype(BF16)
    w2t = np.ascontiguousarray(np.asarray(w2, dtype=np.float32).T).astype(BF16)
    b1c = np.ascontiguousarray(
        np.asarray(b1, dtype=np.float32).reshape(P, 1))
    b2c = np.ascontiguousarray(
        np.asarray(b2, dtype=np.float32).reshape(P, 1))

    vocab = c2e_weight.shape[0]
    assert vocab == VOCAB, vocab

    used, inv = np.unique(nodes, return_inverse=True)
    tot = BLOCKS * P * N_CORES
    rows = BLOCKS * P

    if len(used) <= tot:
        # compact path: transform only referenced rows
        used_pad = np.empty(tot, dtype=np.int64)
        used_pad[:len(used)] = used
        used_pad[len(used):] = int(used[-1]) if len(used) else 0
        compact = c2e_weight[used_pad].astype(BF16)   # [tot, 128]

        common = {"w1t": w1t, "w2t": w2t, "b1c": b1c, "b2c": b2c}
        in_maps = []
        for i in range(N_CORES):
            sl = compact[i * rows:(i + 1) * rows]
            in_maps.append(
                {"tslice": np.ascontiguousarray(sl.T), **common})

        res = _run(_get_nc(BLOCKS), in_maps)

        t2u = np.concatenate(
            [np.asarray(res.results[i]["out"]).T for i in range(N_CORES)],
            axis=0)                                   # [tot, 128] bf16
        return t2u[inv].astype(np.float32)

    # fallback: transform the whole vocab with staggered windows
    frows = FULL_BLOCKS * P
    tableT = np.ascontiguousarray(c2e_weight.T).astype(BF16)  # [128, vocab]
    common = {"w1t": w1t, "w2t": w2t, "b1c": b1c, "b2c": b2c}
    starts, in_maps = [], []
    rng = vocab // N_CORES
    for i in range(N_CORES):
        start = min(i * rng, vocab - frows)
        starts.append(start)
        in_maps.append(
            {"tslice": np.ascontiguousarray(tableT[:, start:start + frows]),
             **common})

    res = _run(_get_nc(FULL_BLOCKS), in_maps)

    t2 = np.empty((vocab, D), dtype=np.float32)
    for i in range(N_CORES):
        dense = np.asarray(res.results[i]["out"])     # [128, frows]
        lo = i * rng
        hi = min((i + 1) * rng, vocab)
        t2[lo:hi] = dense[:, lo - starts[i]:hi - starts[i]].T
    return t2[nodes]


# revision 11
# speedup vs baseline: 2.0032x; 1.0904x over previous
"""Trainium2 Bass kernel for the Context Encoder problem:

    ce  = c2e_weight[nodes]            # [N, 128] embedding gather
    h   = relu(ce @ w1.T + b1)         # [N, 128]
    out = relu(h @ w2.T + b2)          # [N, 128]

Strategy (8 NeuronCores, unique-row compaction, bf16):
  200000 node ids cover ~86.4% of the 100000-row vocab, so transforming
  the table rows that are actually referenced is less total work than
  gathering per-node rows (the output of a per-node gather would be 2.3x
  larger than the compacted table).

  - Host computes used = unique(nodes) (~86.4k rows), pads to a fixed
    8*10880 = 87040 rows, gathers those table rows, downcasts to bf16
    and hands core i its contiguous d-major window [128, 10880].
  - The MLP weights and biases ride as a 258-column bf16 prefix of the
    same stream (separate [128, small] constant DMAs cost ~6us: 128
    tiny descriptors each paying the ~0.8us HBM round trip).
  - All input chunk DMAs are issued up-front on the sync HWDGE ring so
    the reads stream at full bandwidth, decoupled from compute; output
    DMAs follow on the same ring (no head-of-line risk once every
    input is already queued).
  - Each core streams its window through the 2-layer MLP: matmuls in
    bf16 (1 cycle/row on PE vs 4 for fp32), fp32 PSUM accumulation,
    relu+bias fused on ACT/DVE draining PSUM, bf16 results DMA'd out.
    Within a chunk the two layers run as phases; matmuls are 512 wide
    (one fp32 PSUM bank), relu+bias ops 1024 wide over double-buffered
    2-bank PSUM tiles, and the two relu stages alternate between ACT
    and DVE by chunk parity for balance.
  - Host upcasts and applies out = T2used[inverse(nodes)] as the
    unshard step.  If a pathological input references more than 87040
    distinct rows, a full-vocab variant (98 blocks/core, staggered
    windows) is built and used instead.
"""

import os
import sys

for _p in ("/opt/trn_rl_repo",):
    if _p not in sys.path:
        sys.path.insert(0, _p)

import numpy as np
import ml_dtypes

import concourse.bass as bass
import concourse.mybir as mybir
import concourse.tile as tile
from concourse import bacc
from concourse.bass_utils import run_bass_kernel_spmd
from concourse.tile import TileContext

P = 128
D = 128
N_CORES = 8
VOCAB = 100000
BLOCKS = 85                # vocab blocks (of 128 rows) per core, compact path
FULL_BLOCKS = 98           # fallback: cover the whole vocab with overlap
CHUNK = 2048               # columns per input chunk / out DMA
RELU_W = 1024              # columns per relu op (2 fp32 PSUM banks)
FW = 512                   # matmul free width (one fp32 PSUM bank)
PREFIX = 2 * D + 64        # packed w1t | w2t | b1 | b2 | pad ahead of the
                           # data (pad keeps the data start 128B-aligned)

BF16 = ml_dtypes.bfloat16


def build_nc(blocks: int):
    f32 = mybir.dt.float32
    bf16 = mybir.dt.bfloat16
    nc = bacc.Bacc("TRN2", target_bir_lowering=False, debug=False,
                   num_devices=N_CORES)

    rows = blocks * P
    tsl_t = nc.dram_tensor("tslice", [P, PREFIX + rows], bf16,
                           kind="ExternalInput").ap()
    out_t = nc.dram_tensor("out", [P, rows], bf16,
                           kind="ExternalOutput").ap()

    # data-column chunks: small first chunk so compute starts early
    chunks = [RELU_W]
    while sum(chunks) + CHUNK <= rows:
        chunks.append(CHUNK)
    rem = rows - sum(chunks)
    assert rem % P == 0
    if rem:
        chunks.append(rem)

    def pieces(cb, w):
        off = 0
        while off < cb:
            yield off, min(w, cb - off)
            off += w

    with TileContext(nc) as tc:
        with (
            tc.tile_pool(name="const", bufs=1) as cpool,
            tc.tile_pool(name="win0", bufs=1) as g0pool,
            tc.tile_pool(name="win", bufs=len(chunks) - 1) as gpool,
            tc.tile_pool(name="work", bufs=3) as wpool,
            tc.tile_pool(name="obuf", bufs=3) as opool,
            tc.tile_pool(name="psum", bufs=2, space="PSUM") as ppool,
        ):
            # issue every input chunk DMA up-front: the sync ring streams
            # the whole window (constants prefix included) at full rate
            win0 = g0pool.tile([P, PREFIX + RELU_W], bf16, tag="win0")
            nc.sync.dma_start(out=win0[:], in_=tsl_t[:, :PREFIX + RELU_W])
            wins = [(win0, PREFIX)]
            r0 = PREFIX + RELU_W
            for cb in chunks[1:]:
                win = gpool.tile([P, CHUNK], bf16, tag="win")
                nc.sync.dma_start(out=win[:, :cb], in_=tsl_t[:, r0:r0 + cb])
                wins.append((win, 0))
                r0 += cb

            w1t_sb = win0[:, 0:D]
            w2t_sb = win0[:, D:2 * D]
            # stage the biases to fp32 once (GpSimd is otherwise idle);
            # ACT/DVE then read a plain fp32 per-partition scalar
            bias_f32 = cpool.tile([P, 2], f32, tag="bias")
            nc.gpsimd.tensor_scalar_add(
                out=bias_f32[:], in0=win0[:, 2 * D:2 * D + 2], scalar1=0.0)
            b1_sb = bias_f32[:, 0:1]
            b2_sb = bias_f32[:, 1:2]

            def relu_bias(out_ap, in_ap, bias_ap, on_act: bool):
                if on_act:
                    nc.scalar.activation(out_ap, in_ap,
                                         mybir.ActivationFunctionType.Relu,
                                         bias=bias_ap)
                else:
                    nc.vector.tensor_scalar(
                        out=out_ap, in0=in_ap, scalar1=bias_ap,
                        scalar2=0.0, op0=mybir.AluOpType.add,
                        op1=mybir.AluOpType.max)

            r0 = 0
            for ci, cb in enumerate(chunks):
                win, base = wins[ci]
                par = ci % 2 == 0

                # layer 1 phase: relu per RELU_W group
                hts = []
                for goff, gw in pieces(cb, RELU_W):
                    h_ps = ppool.tile([P, RELU_W], f32, tag="h")
                    for off, w in pieces(gw, FW):
                        nc.tensor.matmul(
                            out=h_ps[:, off:off + w],
                            lhsT=w1t_sb,
                            rhs=win[:, base + goff + off:base + goff + off + w],
                            start=True, stop=True)
                    hT = wpool.tile([P, RELU_W], bf16, tag="hT")
                    relu_bias(hT[:, :gw], h_ps[:, :gw], b1_sb, par)
                    hts.append((hT, gw))

                # layer 2 phase
                ob = opool.tile([P, CHUNK], bf16, tag="ob")
                for (hT, gw), (goff, _) in zip(hts, pieces(cb, RELU_W)):
                    o_ps = ppool.tile([P, RELU_W], f32, tag="o")
                    for off, w in pieces(gw, FW):
                        nc.tensor.matmul(
                            out=o_ps[:, off:off + w],
                            lhsT=w2t_sb,
                            rhs=hT[:, off:off + w],
                            start=True, stop=True)
                    relu_bias(ob[:, goff:goff + gw], o_ps[:, :gw], b2_sb,
                              not par)

                nc.sync.dma_start(out=out_t[:, r0:r0 + cb], in_=ob[:, :cb])
                r0 += cb

    nc.compile()
    return nc


_CACHED_NC = {}
LAST_RESULTS = None


def _get_nc(blocks: int):
    nc = _CACHED_NC.get(blocks)
    if nc is None:
        nc = _CACHED_NC[blocks] = build_nc(blocks)
    return nc


def _run(nc, in_maps):
    global LAST_RESULTS
    trace = os.environ.get("BASS_KERNEL_TRACE") == "1"
    if trace:
        try:  # tracing needs the NTFF hook; degrade silently without it
            import antenv.axon_hooks  # noqa: F401
        except ImportError:
            trace = False
    res = run_bass_kernel_spmd(nc, in_maps, core_ids=list(range(N_CORES)),
                               trace=trace)
    LAST_RESULTS = res
    return res


def _prefix_block(w1, b1, w2, b2):
    w1t = np.ascontiguousarray(np.asarray(w1, dtype=np.float32).T)
    w2t = np.ascontiguousarray(np.asarray(w2, dtype=np.float32).T)
    b1c = np.asarray(b1, dtype=np.float32).reshape(P, 1)
    b2c = np.asarray(b2, dtype=np.float32).reshape(P, 1)
    pad = np.zeros((P, PREFIX - 2 * D - 2), dtype=np.float32)
    return np.concatenate([w1t, w2t, b1c, b2c, pad], axis=1).astype(BF16)


def kernel(nodes, c2e_weight, w1, b1, w2, b2):
    nodes = np.asarray(nodes)
    c2e_weight = np.asarray(c2e_weight, dtype=np.float32)
    prefix = _prefix_block(w1, b1, w2, b2)            # [128, PREFIX] bf16

    vocab = c2e_weight.shape[0]
    assert vocab == VOCAB, vocab

    used, inv = np.unique(nodes, return_inverse=True)
    tot = BLOCKS * P * N_CORES
    rows = BLOCKS * P

    if len(used) <= tot:
        # compact path: transform only referenced rows
        used_pad = np.empty(tot, dtype=np.int64)
        used_pad[:len(used)] = used
        used_pad[len(used):] = int(used[-1]) if len(used) else 0
        compact = c2e_weight[used_pad].astype(BF16)   # [tot, 128]

        in_maps = []
        for i in range(N_CORES):
            sl = compact[i * rows:(i + 1) * rows]
            in_maps.append({"tslice": np.ascontiguousarray(
                np.concatenate([prefix, sl.T], axis=1))})

        res = _run(_get_nc(BLOCKS), in_maps)

        t2u = np.concatenate(
            [np.asarray(res.results[i]["out"]).T for i in range(N_CORES)],
            axis=0)                                   # [tot, 128] bf16
        return t2u[inv].astype(np.float32)

    # fallback: transform the whole vocab with staggered windows
    frows = FULL_BLOCKS * P
    tableT = np.ascontiguousarray(c2e_weight.T).astype(BF16)  # [128, vocab]
    starts, in_maps = [], []
    rng = vocab // N_CORES
    for i in range(N_CORES):
        start = min(i * rng, vocab - frows)
        starts.append(start)
        in_maps.append({"tslice": np.ascontiguousarray(
            np.concatenate([prefix, tableT[:, start:start + frows]], axis=1))})

    res = _run(_get_nc(FULL_BLOCKS), in_maps)

    t2 = np.empty((vocab, D), dtype=np.float32)
    for i in range(N_CORES):
        dense = np.asarray(res.results[i]["out"])     # [128, frows]
        lo = i * rng
        hi = min((i + 1) * rng, vocab)
        t2[lo:hi] = dense[:, lo - starts[i]:hi - starts[i]].T
    return t2[nodes]


# revision 12
# speedup vs baseline: 2.1083x; 1.0525x over previous
"""Trainium2 Bass kernel for the Context Encoder problem:

    ce  = c2e_weight[nodes]            # [N, 128] embedding gather
    h   = relu(ce @ w1.T + b1)         # [N, 128]
    out = relu(h @ w2.T + b2)          # [N, 128]

Strategy (8 NeuronCores, unique-row compaction, bf16):
  200000 node ids cover ~86.4% of the 100000-row vocab, so transforming
  the table rows that are actually referenced is less total work than
  gathering per-node rows (the output of a per-node gather would be 2.3x
  larger than the compacted table).

  - Host computes used = unique(nodes) (~86.4k rows), pads to a fixed
    8*10880 = 87040 rows, gathers those table rows, downcasts to bf16
    and hands core i its contiguous d-major window [128, 10880].
  - The MLP weights and biases ride as a 258-column bf16 prefix of the
    same stream (separate [128, small] constant DMAs cost ~6us: 128
    tiny descriptors each paying the ~0.8us HBM round trip).
  - All input chunk DMAs are issued up-front on the sync HWDGE ring so
    the reads stream at full bandwidth, decoupled from compute; output
    DMAs follow on the same ring (no head-of-line risk once every
    input is already queued).
  - Each core streams its window through the 2-layer MLP: matmuls in
    bf16 (1 cycle/row on PE vs 4 for fp32), fp32 PSUM accumulation,
    relu+bias fused on ACT/DVE draining PSUM, bf16 results DMA'd out.
    Within a chunk the two layers run as phases; matmuls are 512 wide
    (one fp32 PSUM bank), relu+bias ops 1024 wide over double-buffered
    2-bank PSUM tiles, and the two relu stages alternate between ACT
    and DVE by chunk parity for balance.
  - Host upcasts and applies out = T2used[inverse(nodes)] as the
    unshard step.  If a pathological input references more than 87040
    distinct rows, a full-vocab variant (98 blocks/core, staggered
    windows) is built and used instead.
"""

import os
import sys

for _p in ("/opt/trn_rl_repo",):
    if _p not in sys.path:
        sys.path.insert(0, _p)

import numpy as np
import ml_dtypes

import concourse.bass as bass
import concourse.mybir as mybir
import concourse.tile as tile
from concourse import bacc
from concourse.bass_utils import run_bass_kernel_spmd
from concourse.tile import TileContext

P = 128
D = 128
N_CORES = 8
VOCAB = 100000
BLOCKS = 85                # vocab blocks (of 128 rows) per core, compact path
FULL_BLOCKS = 98           # fallback: cover the whole vocab with overlap
CHUNK = 2048               # columns per input chunk / out DMA
RELU_W = 1024              # columns per relu op (2 fp32 PSUM banks)
FW = 512                   # matmul free width (one fp32 PSUM bank)
PREFIX = 2 * D + 64        # packed w1t | w2t | b1 | b2 | pad ahead of the
                           # data (pad keeps the data start 128B-aligned)

BF16 = ml_dtypes.bfloat16


def build_nc(blocks: int):
    f32 = mybir.dt.float32
    bf16 = mybir.dt.bfloat16
    nc = bacc.Bacc("TRN2", target_bir_lowering=False, debug=False,
                   num_devices=N_CORES)

    rows = blocks * P
    tsl_t = nc.dram_tensor("tslice", [P, PREFIX + rows], bf16,
                           kind="ExternalInput").ap()
    out_t = nc.dram_tensor("out", [P, rows], bf16,
                           kind="ExternalOutput").ap()

    # data-column chunks: small first chunk so compute starts early
    chunks = [RELU_W]
    while sum(chunks) + CHUNK <= rows:
        chunks.append(CHUNK)
    rem = rows - sum(chunks)
    assert rem % P == 0
    if rem:
        chunks.append(rem)

    def pieces(cb, w):
        off = 0
        while off < cb:
            yield off, min(w, cb - off)
            off += w

    with TileContext(nc) as tc:
        with (
            tc.tile_pool(name="const", bufs=1) as cpool,
            tc.tile_pool(name="win0", bufs=1) as g0pool,
            tc.tile_pool(name="win", bufs=len(chunks) - 1) as gpool,
            tc.tile_pool(name="work", bufs=3) as wpool,
            tc.tile_pool(name="obuf", bufs=3) as opool,
            tc.tile_pool(name="psum", bufs=2, space="PSUM") as ppool,
        ):
            # PE p-state warmup: the HAM throttle only unlocks 2.4 GHz after
            # ~3.4us of sustained activity, and the PE would otherwise idle
            # for the whole DMA-issue window.  Run throwaway matmuls on a
            # memset tile (and one activation, which also hoists the ACT
            # table load) while the input stream is still in flight.
            dummy = cpool.tile([P, FW], bf16, tag="warm")
            nc.gpsimd.memset(dummy[:], 0.0)
            sink = wpool.tile([P, RELU_W], bf16, tag="hT")
            for wi in range(4):
                warm_ps = ppool.tile([P, RELU_W], f32, tag="h")
                for off in (0, FW):
                    nc.tensor.matmul(out=warm_ps[:, off:off + FW],
                                     lhsT=dummy[:, 0:D], rhs=dummy[:],
                                     start=True, stop=True)
                if wi == 0:
                    nc.scalar.activation(sink[:], warm_ps[:],
                                         mybir.ActivationFunctionType.Relu)

            # issue every input chunk DMA up-front: the sync ring streams
            # the whole window (constants prefix included) at full rate
            win0 = g0pool.tile([P, PREFIX + RELU_W], bf16, tag="win0")
            nc.sync.dma_start(out=win0[:], in_=tsl_t[:, :PREFIX + RELU_W])
            wins = [(win0, PREFIX)]
            r0 = PREFIX + RELU_W
            for cb in chunks[1:]:
                win = gpool.tile([P, CHUNK], bf16, tag="win")
                nc.sync.dma_start(out=win[:, :cb], in_=tsl_t[:, r0:r0 + cb])
                wins.append((win, 0))
                r0 += cb

            w1t_sb = win0[:, 0:D]
            w2t_sb = win0[:, D:2 * D]
            # stage the biases to fp32 once (GpSimd is otherwise idle);
            # ACT/DVE then read a plain fp32 per-partition scalar
            bias_f32 = cpool.tile([P, 2], f32, tag="bias")
            nc.gpsimd.tensor_scalar_add(
                out=bias_f32[:], in0=win0[:, 2 * D:2 * D + 2], scalar1=0.0)
            b1_sb = bias_f32[:, 0:1]
            b2_sb = bias_f32[:, 1:2]

            def relu_bias(out_ap, in_ap, bias_ap, on_act: bool):
                if on_act:
                    nc.scalar.activation(out_ap, in_ap,
                                         mybir.ActivationFunctionType.Relu,
                                         bias=bias_ap)
                else:
                    nc.vector.tensor_scalar(
                        out=out_ap, in0=in_ap, scalar1=bias_ap,
                        scalar2=0.0, op0=mybir.AluOpType.add,
                        op1=mybir.AluOpType.max)

            r0 = 0
            for ci, cb in enumerate(chunks):
                win, base = wins[ci]
                par = ci % 2 == 0

                # layer 1 phase: relu per RELU_W group
                hts = []
                for goff, gw in pieces(cb, RELU_W):
                    h_ps = ppool.tile([P, RELU_W], f32, tag="h")
                    for off, w in pieces(gw, FW):
                        nc.tensor.matmul(
                            out=h_ps[:, off:off + w],
                            lhsT=w1t_sb,
                            rhs=win[:, base + goff + off:base + goff + off + w],
                            start=True, stop=True)
                    hT = wpool.tile([P, RELU_W], bf16, tag="hT")
                    relu_bias(hT[:, :gw], h_ps[:, :gw], b1_sb, par)
                    hts.append((hT, gw))

                # layer 2 phase
                ob = opool.tile([P, CHUNK], bf16, tag="ob")
                for (hT, gw), (goff, _) in zip(hts, pieces(cb, RELU_W)):
                    o_ps = ppool.tile([P, RELU_W], f32, tag="o")
                    for off, w in pieces(gw, FW):
                        nc.tensor.matmul(
                            out=o_ps[:, off:off + w],
                            lhsT=w2t_sb,
                            rhs=hT[:, off:off + w],
                            start=True, stop=True)
                    relu_bias(ob[:, goff:goff + gw], o_ps[:, :gw], b2_sb,
                              not par)

                nc.sync.dma_start(out=out_t[:, r0:r0 + cb], in_=ob[:, :cb])
                r0 += cb

    nc.compile()
    return nc


_CACHED_NC = {}
LAST_RESULTS = None


def _get_nc(blocks: int):
    nc = _CACHED_NC.get(blocks)
    if nc is None:
        nc = _CACHED_NC[blocks] = build_nc(blocks)
    return nc


def _run(nc, in_maps):
    global LAST_RESULTS
    trace = os.environ.get("BASS_KERNEL_TRACE") == "1"
    if trace:
        try:  # tracing needs the NTFF hook; degrade silently without it
            import antenv.axon_hooks  # noqa: F401
        except ImportError:
            trace = False
    res = run_bass_kernel_spmd(nc, in_maps, core_ids=list(range(N_CORES)),
                               trace=trace)
    LAST_RESULTS = res
    return res


def _prefix_block(w1, b1, w2, b2):
    w1t = np.ascontiguousarray(np.asarray(w1, dtype=np.float32).T)
    w2t = np.ascontiguousarray(np.asarray(w2, dtype=np.float32).T)
    b1c = np.asarray(b1, dtype=np.float32).reshape(P, 1)
    b2c = np.asarray(b2, dtype=np.float32).reshape(P, 1)
    pad = np.zeros((P, PREFIX - 2 * D - 2), dtype=np.float32)
    return np.concatenate([w1t, w2t, b1c, b2c, pad], axis=1).astype(BF16)


def kernel(nodes, c2e_weight, w1, b1, w2, b2):
    nodes = np.asarray(nodes)
    c2e_weight = np.asarray(c2e_weight, dtype=np.float32)
    prefix = _prefix_block(w1, b1, w2, b2)            # [128, PREFIX] bf16

    vocab = c2e_weight.shape[0]
    assert vocab == VOCAB, vocab

    used, inv = np.unique(nodes, return_inverse=True)
    tot = BLOCKS * P * N_CORES
    rows = BLOCKS * P

    if len(used) <= tot:
        # compact path: transform only referenced rows
        used_pad = np.empty(tot, dtype=np.int64)
        used_pad[:len(used)] = used
        used_pad[len(used):] = int(used[-1]) if len(used) else 0
        compact = c2e_weight[used_pad].astype(BF16)   # [tot, 128]

        in_maps = []
        for i in range(N_CORES):
            sl = compact[i * rows:(i + 1) * rows]
            in_maps.append({"tslice": np.ascontiguousarray(
                np.concatenate([prefix, sl.T], axis=1))})

        res = _run(_get_nc(BLOCKS), in_maps)

        t2u = np.concatenate(
            [np.asarray(res.results[i]["out"]).T for i in range(N_CORES)],
            axis=0)                                   # [tot, 128] bf16
        return t2u[inv].astype(np.float32)

    # fallback: transform the whole vocab with staggered windows
    frows = FULL_BLOCKS * P
    tableT = np.ascontiguousarray(c2e_weight.T).astype(BF16)  # [128, vocab]
    starts, in_maps = [], []
    rng = vocab // N_CORES
    for i in range(N_CORES):
        start = min(i * rng, vocab - frows)
        starts.append(start)
        in_maps.append({"tslice": np.ascontiguousarray(
            np.concatenate([prefix, tableT[:, start:start + frows]], axis=1))})

    res = _run(_get_nc(FULL_BLOCKS), in_maps)

    t2 = np.empty((vocab, D), dtype=np.float32)
    for i in range(N_CORES):
        dense = np.asarray(res.results[i]["out"])     # [128, frows]
        lo = i * rng
        hi = min((i + 1) * rng, vocab)
        t2[lo:hi] = dense[:, lo - starts[i]:hi - starts[i]].T
    return t2[nodes]
